# revision 3
# baseline (speedup 1.0000x reference)
"""Distributed attention kernel for 8 Trainium2 NeuronCores.

Computes reference:
    q = Q @ Wq.T ; k = K @ Wk.T ; v = V @ Wv.T
    out = softmax((q @ k.T) / sqrt(din)) @ v
with N=4096, DIN=DOUT=1024, fp32 inputs/outputs.

Sharding: rows of Q/K/V are split 512/core.  Each core computes its own
q.T, k.T and v shards (bf16), AllGathers k.T and v, then does its block of
rows of the attention.  All matmuls run with the contraction dim on the
partition axis, so inputs/weights are PE-transposed (fp32 transpose via
identity matmul, cast to bf16 on the PSUM->SBUF copy).  Softmax runs in
transposed layout [l, i] (keys on partitions): exp on ScalarE without
max-subtraction (logits are O(5) here), row-denominators via N=1
ones-matmuls sharing the p.T stationary tiles.
"""

import sys

sys.path.insert(0, "/opt/trn_rl_repo")

import json

import numpy as np

import concourse.bass as bass
import concourse.bass2jax as bass2jax
import concourse.bass_utils as bass_utils
import concourse.mybir as mybir
import concourse.tile as tile
from concourse.masks import make_identity

N_CORES = 8
N = 4096
D = 1024
NS = N // N_CORES          # 512 rows per core
P = 128                    # partitions
NT = NS // P               # 4 row-tiles per shard
DT = D // P                # 8 feature tiles
LT = N // P                # 32 key tiles global
F32 = mybir.dt.float32
BF16 = mybir.dt.bfloat16

# ---------------------------------------------------------------------------
# walrus compat: this container's walrus rejects >1 sync wait per instruction.
# Rewrite the BIR before compiling: extra waits become wait-only NoOps on the
# same engine immediately before the instruction.  Safe because Tile assigns
# waits against a global instruction order (waits only reference earlier
# instructions), so engine-blocking earlier only adds stalls, never cycles.
# ---------------------------------------------------------------------------
_orig_compile_bir_kernel = bass_utils.compile_bir_kernel


def _split_waits(mod):
    ctr = 0
    for func in mod.get("functions", []):
        for blk in func.get("blocks", []):
            insts = blk.get("instructions", [])
            if not any(
                len((i.get("sync_info") or {}).get("on_wait") or []) > 1
                for i in insts
            ):
                continue
            new_insts = []
            for ins in insts:
                si = ins.get("sync_info")
                waits = (si or {}).get("on_wait") or []
                if len(waits) > 1:
                    for w in waits[:-1]:
                        ctr += 1
                        new_insts.append(
                            {
                                "debug": ins.get("debug", 0),
                                "engine": ins["engine"],
                                "ins": [],
                                "outs": [],
                                "name": f"{ins['name']}_sw{ctr}",
                                "opcode": "NoOp",
                                "sync_info": {"on_wait": [w], "on_update": []},
                            }
                        )
                    si["on_wait"] = [waits[-1]]
                new_insts.append(ins)
            blk["instructions"] = new_insts
    return ctr


def _patched_compile_bir_kernel(bir_json, tmpdir, neff_name="file.neff"):
    mod = json.loads(bir_json)
    if _split_waits(mod):
        bir_json = json.dumps(mod).encode()
    return _orig_compile_bir_kernel(bir_json, tmpdir, neff_name)


bass_utils.compile_bir_kernel = _patched_compile_bir_kernel
bass2jax.compile_bir_kernel = _patched_compile_bir_kernel


# ---------------------------------------------------------------------------
# kernel build
# ---------------------------------------------------------------------------
def _transpose_cast(nc, ps_pool, dst, src_ap, copy_engine, ident):
    """PE-transpose one [128,128] fp32 tile and cast-copy it into dst (bf16)."""
    pt = ps_pool.tile([P, P], F32, tag="tp")
    nc.tensor.transpose(pt[:], src_ap, ident[:])
    if copy_engine == "v":
        nc.vector.tensor_copy(out=dst, in_=pt[:])
    else:
        nc.scalar.copy(out=dst, in_=pt[:])


def build_nc():
    nc = bass.Bass(num_devices=N_CORES)

    Qp = nc.declare_dram_parameter("q_in", [NS, D], F32, isOutput=False)
    Kp = nc.declare_dram_parameter("k_in", [NS, D], F32, isOutput=False)
    Vp = nc.declare_dram_parameter("v_in", [NS, D], F32, isOutput=False)
    Wqp = nc.declare_dram_parameter("wq", [D, D], F32, isOutput=False)
    Wkp = nc.declare_dram_parameter("wk", [D, D], F32, isOutput=False)
    Wvp = nc.declare_dram_parameter("wv", [D, D], F32, isOutput=False)
    out_p = nc.declare_dram_parameter("out", [NS, D], F32, isOutput=True)

    # internal DRAM for collectives
    cc_k_in = nc.dram_tensor("cc_k_in", [DT, P, NS], BF16)
    cc_k_out = nc.dram_tensor("cc_k_out", [N_CORES, DT, P, NS], BF16,
                              addr_space="Shared")
    cc_v_in = nc.dram_tensor("cc_v_in", [NT, P, D], BF16)
    cc_v_out = nc.dram_tensor("cc_v_out", [N_CORES, NT, P, D], BF16,
                              addr_space="Shared")

    with tile.TileContext(nc) as tc:
        with tc.tile_pool(name="persist", bufs=1) as pp:
            ident = pp.tile([P, P], F32)
            make_identity(nc, ident[:])
            ones = pp.tile([P, 1], BF16)
            nc.vector.memset(ones[:], 1.0)
            # persistent bf16 arrays
            qT = pp.tile([P, DT, NS], BF16)      # q.T  [j, i]
            pT = pp.tile([P, LT, NS], BF16)      # softmax numerators [l, i]

            with tc.tile_pool(name="ps_stage", bufs=4, space="PSUM") as psst, \
                 tc.tile_pool(name="stage", bufs=3) as stg:

                def load_transpose(param, n_row_tiles, dst, eng):
                    # param: DRAM [n_row_tiles*P, D] fp32; dst bf16 tile
                    # [P, DT, n_row_tiles*P] holding param.T ([d, row]).
                    for rb in range(n_row_tiles):
                        s = stg.tile([P, D], F32, tag="ldw")
                        nc.sync.dma_start(
                            out=s[:], in_=param[rb * P:(rb + 1) * P, :]
                        )
                        for dt in range(DT):
                            _transpose_cast(
                                nc, psst,
                                dst[:, dt, rb * P:(rb + 1) * P],
                                s[:, dt * P:(dt + 1) * P],
                                eng, ident,
                            )

                # --- K branch first so its collective launches earliest ---
                wkT = stg.tile([P, DT, D], BF16, tag="wkT")
                load_transpose(Wkp, DT, wkT, "v")
                kTl = stg.tile([P, DT, NS], BF16, tag="kTl")
                load_transpose(Kp, NT, kTl, "s")
                # kT_loc[j, l_loc] = sum_d WkT[d, j-slice].T @ KT[d, l_loc]
                kTs = stg.tile([P, DT, NS], BF16, tag="kTs")
                for jt in range(DT):
                    pk = psst.tile([P, NS], F32, tag="mm")
                    for dt in range(DT):
                        nc.tensor.matmul(
                            pk[:],
                            wkT[:, dt, jt * P:(jt + 1) * P],
                            kTl[:, dt, :],
                            start=(dt == 0), stop=(dt == DT - 1),
                        )
                    nc.vector.tensor_copy(out=kTs[:, jt, :], in_=pk[:])
                    nc.sync.dma_start(out=cc_k_in[jt], in_=kTs[:, jt, :])
                nc.gpsimd.collective_compute(
                    "AllGather", mybir.AluOpType.bypass,
                    replica_groups=[list(range(N_CORES))],
                    ins=[cc_k_in[:]], outs=[cc_k_out[:]],
                )

                # --- V branch ---
                wvT = stg.tile([P, DT, D], BF16, tag="wkT")
                load_transpose(Wvp, DT, wvT, "v")
                vTl = stg.tile([P, DT, NS], BF16, tag="kTl")
                load_transpose(Vp, NT, vTl, "s")
                # v_loc[l_loc, m] = sum_d VT[d, l-slice].T @ WvT[d, m]
                vls = stg.tile([P, NT, D], BF16, tag="vls")
                for lt in range(NT):
                    for mh in range(2):
                        pv = psst.tile([P, NS], F32, tag="mm")
                        for dt in range(DT):
                            nc.tensor.matmul(
                                pv[:],
                                vTl[:, dt, lt * P:(lt + 1) * P],
                                wvT[:, dt, mh * NS:(mh + 1) * NS],
                                start=(dt == 0), stop=(dt == DT - 1),
                            )
                        nc.vector.tensor_copy(
                            out=vls[:, lt, mh * NS:(mh + 1) * NS], in_=pv[:]
                        )
                    nc.sync.dma_start(out=cc_v_in[lt], in_=vls[:, lt, :])
                nc.gpsimd.collective_compute(
                    "AllGather", mybir.AluOpType.bypass,
                    replica_groups=[list(range(N_CORES))],
                    ins=[cc_v_in[:]], outs=[cc_v_out[:]],
                )

                # --- Q branch (local only; overlaps the collectives) ---
                wqT = stg.tile([P, DT, D], BF16, tag="wkT")
                load_transpose(Wqp, DT, wqT, "v")
                qTl = stg.tile([P, DT, NS], BF16, tag="kTl")
                load_transpose(Qp, NT, qTl, "s")
                for jt in range(DT):
                    pq = psst.tile([P, NS], F32, tag="mm")
                    for dt in range(DT):
                        nc.tensor.matmul(
                            pq[:],
                            wqT[:, dt, jt * P:(jt + 1) * P],
                            qTl[:, dt, :],
                            start=(dt == 0), stop=(dt == DT - 1),
                        )
                    nc.vector.tensor_copy(out=qT[:, jt, :], in_=pq[:])

            # stage pool freed; bring in gathered k.T / v
            with tc.tile_pool(name="gathered", bufs=1) as gp, \
                 tc.tile_pool(name="ps_main", bufs=1, space="PSUM") as psm:
                kT = gp.tile([P, DT, N], BF16)    # k.T [j, l] full
                vF = gp.tile([P, LT, D], BF16)    # v  [l, m] full
                for jt in range(DT):
                    for r in range(N_CORES):
                        nc.sync.dma_start(
                            out=kT[:, jt, r * NS:(r + 1) * NS],
                            in_=cc_k_out[r, jt],
                        )
                for r in range(N_CORES):
                    for lt in range(NT):
                        nc.sync.dma_start(
                            out=vF[:, r * NT + lt, :],
                            in_=cc_v_out[r, lt],
                        )

                scale = float(1.0 / np.sqrt(D))
                po = [psm.tile([P, NS], F32, tag=f"po{it}", name=f"po{it}")
                      for it in range(NT)]

                # pass A: scores + exp for all l, out-matmuls for m-half 0.
                # NOTE: a matmul's start=True clears has_written for its whole
                # PSUM bank, so each concurrent accumulation chain must own
                # its own tile (tiles are padded to a bank).
                for lt in range(LT):
                    ps = psm.tile([P, NS], F32, tag="scores", bufs=2)
                    for jt in range(DT):
                        nc.tensor.matmul(
                            ps[:],
                            kT[:, jt, lt * P:(lt + 1) * P],
                            qT[:, jt, :],
                            start=(jt == 0), stop=(jt == DT - 1),
                        )
                    nc.scalar.activation(
                        out=pT[:, lt, :], in_=ps[:],
                        func=mybir.ActivationFunctionType.Exp, scale=scale,
                    )
                    for it in range(NT):
                        nc.tensor.matmul(
                            po[it][:],
                            pT[:, lt, it * P:(it + 1) * P],
                            vF[:, lt, 0:NS],
                            start=(lt == 0), stop=(lt == LT - 1),
                            skip_group_check=True,
                        )

                # denominator chains (one PSUM tile each, sequential), then
                # reciprocals; interleaved with pass B below by the scheduler
                dnr = gp.tile([P, NT], F32)
                for it in range(NT):
                    dnp = psm.tile([P, 1], F32, tag="dnc", bufs=2,
                                   name=f"dnp{it}")
                    for lt in range(LT):
                        nc.tensor.matmul(
                            dnp[:],
                            pT[:, lt, it * P:(it + 1) * P],
                            ones[:],
                            start=(lt == 0), stop=(lt == LT - 1),
                            skip_group_check=True,
                        )
                    nc.vector.tensor_copy(out=dnr[:, it:it + 1], in_=dnp[:])
                rec = gp.tile([P, NT], F32)
                nc.vector.reciprocal(out=rec[:], in_=dnr[:])

                # normalize + store m-half 0
                for it in range(NT):
                    ob = gp.tile([P, NS], F32, tag="ob", bufs=2)
                    nc.vector.tensor_scalar_mul(
                        out=ob[:], in0=po[it][:], scalar1=rec[:, it:it + 1]
                    )
                    nc.sync.dma_start(
                        out=out_p[it * P:(it + 1) * P, 0:NS], in_=ob[:]
                    )

                # pass B: out-matmuls for m-half 1
                po2 = [psm.tile([P, NS], F32, tag=f"po{it}", name=f"po2_{it}")
                       for it in range(NT)]
                for lt in range(LT):
                    for it in range(NT):
                        nc.tensor.matmul(
                            po2[it][:],
                            pT[:, lt, it * P:(it + 1) * P],
                            vF[:, lt, NS:D],
                            start=(lt == 0), stop=(lt == LT - 1),
                            skip_group_check=True,
                        )
                for it in range(NT):
                    ob = gp.tile([P, NS], F32, tag="ob", bufs=2)
                    nc.vector.tensor_scalar_mul(
                        out=ob[:], in0=po2[it][:], scalar1=rec[:, it:it + 1]
                    )
                    nc.sync.dma_start(
                        out=out_p[it * P:(it + 1) * P, NS:D], in_=ob[:]
                    )

    return nc


_nc_cache = None


def _get_nc():
    global _nc_cache
    if _nc_cache is None:
        _nc_cache = build_nc()
    return _nc_cache


def kernel(Q, K, V, Wq, Wk, Wv, _trace=False):
    from concourse.bass_utils import run_bass_kernel_spmd

    Q = np.ascontiguousarray(np.asarray(Q, dtype=np.float32))
    K = np.ascontiguousarray(np.asarray(K, dtype=np.float32))
    V = np.ascontiguousarray(np.asarray(V, dtype=np.float32))
    Wq = np.ascontiguousarray(np.asarray(Wq, dtype=np.float32))
    Wk = np.ascontiguousarray(np.asarray(Wk, dtype=np.float32))
    Wv = np.ascontiguousarray(np.asarray(Wv, dtype=np.float32))

    nc = _get_nc()
    in_maps = []
    for c in range(N_CORES):
        sl = slice(c * NS, (c + 1) * NS)
        in_maps.append({
            "q_in": Q[sl], "k_in": K[sl], "v_in": V[sl],
            "wq": Wq, "wk": Wk, "wv": Wv,
        })
    res = run_bass_kernel_spmd(
        nc, in_maps, list(range(N_CORES)), trace=_trace
    )
    out = np.concatenate([res.results[c]["out"] for c in range(N_CORES)], axis=0)
    if _trace:
        kernel.last_exec_time_ns = res.exec_time_ns
        kernel.last_results = res
    return out


# revision 6
# speedup vs baseline: 1.0436x; 1.0436x over previous
"""Distributed attention kernel for 8 Trainium2 NeuronCores.

Computes reference:
    q = Q @ Wq.T ; k = K @ Wk.T ; v = V @ Wv.T
    out = softmax((q @ k.T) / sqrt(din)) @ v
with N=4096, DIN=DOUT=1024, fp32 inputs/outputs.

Sharding: rows of Q/K/V are split 512/core.  Each core computes its own
q.T, k.T and v shards (bf16), AllGathers k.T and v, then does its block of
rows of the attention.  All matmuls run with the contraction dim on the
partition axis, so inputs/weights are PE-transposed (fp32 transpose via
identity matmul, cast to bf16 on the PSUM->SBUF copy).  Softmax runs in
transposed layout [l, i] (keys on partitions): exp on ScalarE without
max-subtraction (logits are O(5) here), row-denominators via N=1
ones-matmuls sharing the p.T stationary tiles.
"""

import sys

sys.path.insert(0, "/opt/trn_rl_repo")

import json

import numpy as np

import concourse.bass as bass
import concourse.bass2jax as bass2jax
import concourse.bass_utils as bass_utils
import concourse.mybir as mybir
import concourse.tile as tile
from concourse.masks import make_identity

N_CORES = 8
N = 4096
D = 1024
NS = N // N_CORES          # 512 rows per core
P = 128                    # partitions
NT = NS // P               # 4 row-tiles per shard
DT = D // P                # 8 feature tiles
LT = N // P                # 32 key tiles global
F32 = mybir.dt.float32
BF16 = mybir.dt.bfloat16

# ---------------------------------------------------------------------------
# walrus compat: this container's walrus rejects >1 sync wait per instruction.
# Rewrite the BIR before compiling: extra waits become wait-only NoOps on the
# same engine immediately before the instruction.  Safe because Tile assigns
# waits against a global instruction order (waits only reference earlier
# instructions), so engine-blocking earlier only adds stalls, never cycles.
# ---------------------------------------------------------------------------
_orig_compile_bir_kernel = bass_utils.compile_bir_kernel


def _split_waits(mod):
    ctr = 0
    for func in mod.get("functions", []):
        for blk in func.get("blocks", []):
            insts = blk.get("instructions", [])
            if not any(
                len((i.get("sync_info") or {}).get("on_wait") or []) > 1
                for i in insts
            ):
                continue
            new_insts = []
            for ins in insts:
                si = ins.get("sync_info")
                waits = (si or {}).get("on_wait") or []
                if len(waits) > 1:
                    for w in waits[:-1]:
                        ctr += 1
                        new_insts.append(
                            {
                                "debug": ins.get("debug", 0),
                                "engine": ins["engine"],
                                "ins": [],
                                "outs": [],
                                "name": f"{ins['name']}_sw{ctr}",
                                "opcode": "NoOp",
                                "sync_info": {"on_wait": [w], "on_update": []},
                            }
                        )
                    si["on_wait"] = [waits[-1]]
                new_insts.append(ins)
            blk["instructions"] = new_insts
    return ctr


def _patched_compile_bir_kernel(bir_json, tmpdir, neff_name="file.neff"):
    mod = json.loads(bir_json)
    if _split_waits(mod):
        bir_json = json.dumps(mod).encode()
    return _orig_compile_bir_kernel(bir_json, tmpdir, neff_name)


bass_utils.compile_bir_kernel = _patched_compile_bir_kernel
bass2jax.compile_bir_kernel = _patched_compile_bir_kernel


# ---------------------------------------------------------------------------
# kernel build
# ---------------------------------------------------------------------------
def _transpose_cast(nc, ps_pool, dst, src_ap, copy_engine, ident):
    """PE-transpose one [128,128] fp32 tile and cast-copy it into dst (bf16)."""
    pt = ps_pool.tile([P, P], F32, tag="tp")
    nc.tensor.transpose(pt[:], src_ap, ident[:])
    if copy_engine == "v":
        nc.vector.tensor_copy(out=dst, in_=pt[:])
    else:
        nc.scalar.copy(out=dst, in_=pt[:])


def build_nc():
    nc = bass.Bass(num_devices=N_CORES)

    Qp = nc.declare_dram_parameter("q_in", [NS, D], F32, isOutput=False)
    Kp = nc.declare_dram_parameter("k_in", [NS, D], F32, isOutput=False)
    Vp = nc.declare_dram_parameter("v_in", [NS, D], F32, isOutput=False)
    Wqp = nc.declare_dram_parameter("wq", [D, D], F32, isOutput=False)
    Wkp = nc.declare_dram_parameter("wk", [D, D], F32, isOutput=False)
    Wvp = nc.declare_dram_parameter("wv", [D, D], F32, isOutput=False)
    out_p = nc.declare_dram_parameter("out", [NS, D], F32, isOutput=True)

    # internal DRAM for collectives
    cc_k_in = nc.dram_tensor("cc_k_in", [DT, P, NS], BF16)
    cc_k_out = nc.dram_tensor("cc_k_out", [N_CORES, DT, P, NS], BF16,
                              addr_space="Shared")
    cc_v_in = nc.dram_tensor("cc_v_in", [NT, P, D], BF16)
    cc_v_out = nc.dram_tensor("cc_v_out", [N_CORES, NT, P, D], BF16,
                              addr_space="Shared")
    cc_warm_in = nc.dram_tensor("cc_warm_in", [1, 64], BF16)
    cc_warm_out = nc.dram_tensor("cc_warm_out", [N_CORES, 64], BF16,
                                 addr_space="Shared")

    with tile.TileContext(nc) as tc:
        with tc.tile_pool(name="persist", bufs=1) as pp:
            # tiny warm-up collective issued immediately: the first collective
            # pays a multi-10us comm-init barrier on the CC stream; absorb it
            # while the PE is busy with transposes so the real k-gather
            # starts without delay.
            nc.gpsimd.collective_compute(
                "AllGather", mybir.AluOpType.bypass,
                replica_groups=[list(range(N_CORES))],
                ins=[cc_warm_in[:]], outs=[cc_warm_out[:]],
            )
            ident = pp.tile([P, P], F32)
            make_identity(nc, ident[:])
            ones = pp.tile([P, 1], BF16)
            nc.vector.memset(ones[:], 1.0)
            # persistent bf16 arrays
            qT = pp.tile([P, DT, NS], BF16)      # q.T  [j, i]
            pT = pp.tile([P, LT, NS], BF16)      # softmax numerators [l, i]

            with tc.tile_pool(name="ps_stage", bufs=4, space="PSUM") as psst, \
                 tc.tile_pool(name="stage", bufs=3) as stg:

                def load_transpose(param, n_row_tiles, dst, eng):
                    # param: DRAM [n_row_tiles*P, D] fp32; dst bf16 tile
                    # [P, DT, n_row_tiles*P] holding param.T ([d, row]).
                    for rb in range(n_row_tiles):
                        s = stg.tile([P, D], F32, tag="ldw")
                        nc.sync.dma_start(
                            out=s[:], in_=param[rb * P:(rb + 1) * P, :]
                        )
                        for dt in range(DT):
                            _transpose_cast(
                                nc, psst,
                                dst[:, dt, rb * P:(rb + 1) * P],
                                s[:, dt * P:(dt + 1) * P],
                                eng, ident,
                            )

                # --- K branch first so its collective launches earliest ---
                wkT = stg.tile([P, DT, D], BF16, tag="wkT")
                load_transpose(Wkp, DT, wkT, "v")
                kTl = stg.tile([P, DT, NS], BF16, tag="kTl")
                load_transpose(Kp, NT, kTl, "s")
                # kT_loc[j, l_loc] = sum_d WkT[d, j-slice].T @ KT[d, l_loc]
                kTs = stg.tile([P, DT, NS], BF16, tag="kTs")
                for jt in range(DT):
                    pk = psst.tile([P, NS], F32, tag="mm")
                    for dt in range(DT):
                        nc.tensor.matmul(
                            pk[:],
                            wkT[:, dt, jt * P:(jt + 1) * P],
                            kTl[:, dt, :],
                            start=(dt == 0), stop=(dt == DT - 1),
                        )
                    nc.vector.tensor_copy(out=kTs[:, jt, :], in_=pk[:])
                    nc.sync.dma_start(out=cc_k_in[jt], in_=kTs[:, jt, :])
                nc.gpsimd.collective_compute(
                    "AllGather", mybir.AluOpType.bypass,
                    replica_groups=[list(range(N_CORES))],
                    ins=[cc_k_in[:]], outs=[cc_k_out[:]],
                )

                # --- Q branch (local only; overlaps the k collective) ---
                wqT = stg.tile([P, DT, D], BF16, tag="wkT")
                load_transpose(Wqp, DT, wqT, "v")
                qTl = stg.tile([P, DT, NS], BF16, tag="kTl")
                load_transpose(Qp, NT, qTl, "s")
                for jt in range(DT):
                    pq = psst.tile([P, NS], F32, tag="mm")
                    for dt in range(DT):
                        nc.tensor.matmul(
                            pq[:],
                            wqT[:, dt, jt * P:(jt + 1) * P],
                            qTl[:, dt, :],
                            start=(dt == 0), stop=(dt == DT - 1),
                        )
                    nc.vector.tensor_copy(out=qT[:, jt, :], in_=pq[:])

                # --- V branch ---
                wvT = stg.tile([P, DT, D], BF16, tag="wkT")
                load_transpose(Wvp, DT, wvT, "v")
                vTl = stg.tile([P, DT, NS], BF16, tag="kTl")
                load_transpose(Vp, NT, vTl, "s")
                # v_loc[l_loc, m] = sum_d VT[d, l-slice].T @ WvT[d, m]
                vls = stg.tile([P, NT, D], BF16, tag="vls")
                for lt in range(NT):
                    for mh in range(2):
                        pv = psst.tile([P, NS], F32, tag="mm")
                        for dt in range(DT):
                            nc.tensor.matmul(
                                pv[:],
                                vTl[:, dt, lt * P:(lt + 1) * P],
                                wvT[:, dt, mh * NS:(mh + 1) * NS],
                                start=(dt == 0), stop=(dt == DT - 1),
                            )
                        nc.vector.tensor_copy(
                            out=vls[:, lt, mh * NS:(mh + 1) * NS], in_=pv[:]
                        )
                    nc.sync.dma_start(out=cc_v_in[lt], in_=vls[:, lt, :])
                nc.gpsimd.collective_compute(
                    "AllGather", mybir.AluOpType.bypass,
                    replica_groups=[list(range(N_CORES))],
                    ins=[cc_v_in[:]], outs=[cc_v_out[:]],
                )

            # stage pool freed; bring in gathered k.T / v
            with tc.tile_pool(name="gathered", bufs=1) as gp, \
                 tc.tile_pool(name="ps_po", bufs=1, space="PSUM") as psm_po:
                kT = gp.tile([P, DT, N], BF16)    # k.T [j, l] full
                vF = gp.tile([P, LT, D], BF16)    # v  [l, m] full
                # rank-major so the scores for rank r's key block can start
                # as soon as that rank's slice has landed
                for r in range(N_CORES):
                    for jt in range(DT):
                        nc.sync.dma_start(
                            out=kT[:, jt, r * NS:(r + 1) * NS],
                            in_=cc_k_out[r, jt],
                        )
                for r in range(N_CORES):
                    for lt in range(NT):
                        nc.sync.dma_start(
                            out=vF[:, r * NT + lt, :],
                            in_=cc_v_out[r, lt],
                        )

                scale = float(1.0 / np.sqrt(D))
                po = [psm_po.tile([P, NS], F32, tag=f"po{it}", name=f"po{it}")
                      for it in range(NT)]
                unorm0 = gp.tile([P, NT, NS], F32)   # pass-A output (SBUF)

                # pass A: scores + exp for all l, out-matmuls for m-half 0.
                # NOTE: a matmul's start=True clears has_written for its whole
                # PSUM bank, so each concurrent accumulation chain must own
                # its own tile (tiles are padded to a bank).
                with tc.tile_pool(name="ps_sc", bufs=1, space="PSUM") as psm_sc:
                    for lt in range(LT):
                        ps = psm_sc.tile([P, NS], F32, tag="scores", bufs=2)
                        for jt in range(DT):
                            nc.tensor.matmul(
                                ps[:],
                                kT[:, jt, lt * P:(lt + 1) * P],
                                qT[:, jt, :],
                                start=(jt == 0), stop=(jt == DT - 1),
                            )
                        nc.scalar.activation(
                            out=pT[:, lt, :], in_=ps[:],
                            func=mybir.ActivationFunctionType.Exp, scale=scale,
                        )
                        for it in range(NT):
                            nc.tensor.matmul(
                                po[it][:],
                                pT[:, lt, it * P:(it + 1) * P],
                                vF[:, lt, 0:NS],
                                start=(lt == 0), stop=(lt == LT - 1),
                                skip_group_check=True,
                            )
                    # free po banks for pass B: park pass-A sums in SBUF
                    for it in range(NT):
                        nc.vector.tensor_copy(
                            out=unorm0[:, it, :], in_=po[it][:]
                        )

                # pass B: out-matmuls for m-half 1, denominator chains
                # interleaved (each chain owns a PSUM tile = its own bank,
                # and shares its stationary pT tile with the po2 matmul)
                with tc.tile_pool(name="ps_dn", bufs=1, space="PSUM") as psm_dn:
                    po2 = [psm_po.tile([P, NS], F32, tag=f"po{it}",
                                       name=f"po2_{it}")
                           for it in range(NT)]
                    dn = [psm_dn.tile([P, 1], F32, tag=f"dn{it}",
                                      name=f"dn{it}")
                          for it in range(NT)]
                    for lt in range(LT):
                        for it in range(NT):
                            nc.tensor.matmul(
                                po2[it][:],
                                pT[:, lt, it * P:(it + 1) * P],
                                vF[:, lt, NS:D],
                                start=(lt == 0), stop=(lt == LT - 1),
                                skip_group_check=True,
                            )
                            nc.tensor.matmul(
                                dn[it][:],
                                pT[:, lt, it * P:(it + 1) * P],
                                ones[:],
                                start=(lt == 0), stop=(lt == LT - 1),
                                skip_group_check=True,
                            )
                    dnr = gp.tile([P, NT], F32)
                    for it in range(NT):
                        nc.vector.tensor_copy(
                            out=dnr[:, it:it + 1], in_=dn[it][:]
                        )
                    rec = gp.tile([P, NT], F32)
                    nc.vector.reciprocal(out=rec[:], in_=dnr[:])

                    # normalize + store
                    for it in range(NT):
                        ob = gp.tile([P, NS], F32, tag="ob", bufs=2,
                                     name=f"ob0_{it}")
                        nc.vector.tensor_scalar_mul(
                            out=ob[:], in0=unorm0[:, it, :],
                            scalar1=rec[:, it:it + 1]
                        )
                        nc.sync.dma_start(
                            out=out_p[it * P:(it + 1) * P, 0:NS], in_=ob[:]
                        )
                        ob1 = gp.tile([P, NS], F32, tag="ob", bufs=2,
                                      name=f"ob1_{it}")
                        nc.vector.tensor_scalar_mul(
                            out=ob1[:], in0=po2[it][:],
                            scalar1=rec[:, it:it + 1]
                        )
                        nc.sync.dma_start(
                            out=out_p[it * P:(it + 1) * P, NS:D], in_=ob1[:]
                        )

    return nc


_nc_cache = None


def _get_nc():
    global _nc_cache
    if _nc_cache is None:
        _nc_cache = build_nc()
    return _nc_cache


def kernel(Q, K, V, Wq, Wk, Wv, _trace=False):
    from concourse.bass_utils import run_bass_kernel_spmd

    Q = np.ascontiguousarray(np.asarray(Q, dtype=np.float32))
    K = np.ascontiguousarray(np.asarray(K, dtype=np.float32))
    V = np.ascontiguousarray(np.asarray(V, dtype=np.float32))
    Wq = np.ascontiguousarray(np.asarray(Wq, dtype=np.float32))
    Wk = np.ascontiguousarray(np.asarray(Wk, dtype=np.float32))
    Wv = np.ascontiguousarray(np.asarray(Wv, dtype=np.float32))

    nc = _get_nc()
    in_maps = []
    for c in range(N_CORES):
        sl = slice(c * NS, (c + 1) * NS)
        in_maps.append({
            "q_in": Q[sl], "k_in": K[sl], "v_in": V[sl],
            "wq": Wq, "wk": Wk, "wv": Wv,
        })
    res = run_bass_kernel_spmd(
        nc, in_maps, list(range(N_CORES)), trace=_trace
    )
    out = np.concatenate([res.results[c]["out"] for c in range(N_CORES)], axis=0)
    if _trace:
        kernel.last_exec_time_ns = res.exec_time_ns
        kernel.last_results = res
    return out


# revision 9
# speedup vs baseline: 1.0970x; 1.0512x over previous
"""Distributed attention kernel for 8 Trainium2 NeuronCores.

Computes reference:
    q = Q @ Wq.T ; k = K @ Wk.T ; v = V @ Wv.T
    out = softmax((q @ k.T) / sqrt(din)) @ v
with N=4096, DIN=DOUT=1024, fp32 inputs/outputs.

Sharding: rows of Q/K/V are split 512/core.  Each core computes its own
q.T, k.T and v shards (bf16), AllGathers k.T and v, then does its block of
rows of the attention.  All matmuls run with the contraction dim on the
partition axis, so inputs/weights are PE-transposed (fp32 transpose via
identity matmul, cast to bf16 on the PSUM->SBUF copy).  Softmax runs in
transposed layout [l, i] (keys on partitions): exp on ScalarE without
max-subtraction (logits are O(5) here), row-denominators via N=1
ones-matmuls sharing the p.T stationary tiles.
"""

import sys

sys.path.insert(0, "/opt/trn_rl_repo")

import json

import numpy as np

import concourse.bass as bass
import concourse.bass2jax as bass2jax
import concourse.bass_utils as bass_utils
import concourse.mybir as mybir
import concourse.tile as tile
from concourse.masks import make_identity

N_CORES = 8
N = 4096
D = 1024
NS = N // N_CORES          # 512 rows per core
P = 128                    # partitions
NT = NS // P               # 4 row-tiles per shard
DT = D // P                # 8 feature tiles
LT = N // P                # 32 key tiles global
F32 = mybir.dt.float32
BF16 = mybir.dt.bfloat16

# ---------------------------------------------------------------------------
# walrus compat: this container's walrus rejects >1 sync wait per instruction.
# Rewrite the BIR before compiling: extra waits become wait-only NoOps on the
# same engine immediately before the instruction.  Safe because Tile assigns
# waits against a global instruction order (waits only reference earlier
# instructions), so engine-blocking earlier only adds stalls, never cycles.
# ---------------------------------------------------------------------------
_orig_compile_bir_kernel = bass_utils.compile_bir_kernel


def _split_waits(mod):
    ctr = 0
    for func in mod.get("functions", []):
        for blk in func.get("blocks", []):
            insts = blk.get("instructions", [])
            if not any(
                len((i.get("sync_info") or {}).get("on_wait") or []) > 1
                for i in insts
            ):
                continue
            new_insts = []
            for ins in insts:
                si = ins.get("sync_info")
                waits = (si or {}).get("on_wait") or []
                if len(waits) > 1:
                    for w in waits[:-1]:
                        ctr += 1
                        new_insts.append(
                            {
                                "debug": ins.get("debug", 0),
                                "engine": ins["engine"],
                                "ins": [],
                                "outs": [],
                                "name": f"{ins['name']}_sw{ctr}",
                                "opcode": "NoOp",
                                "sync_info": {"on_wait": [w], "on_update": []},
                            }
                        )
                    si["on_wait"] = [waits[-1]]
                new_insts.append(ins)
            blk["instructions"] = new_insts
    return ctr


def _fix_collective_waits(mod):
    """Replace each collective's scheduler-assigned waits (conservative:
    every DMA queue at its scheduled position) with exactly the completion
    counts of the DMAs that WRITE its input tensor.  The warm-up collective
    (input never written) ends up with no waits and triggers immediately.
    """
    n = 0
    for func in mod.get("functions", []):
        # pass 1: per-semaphore cumulative update counts at each
        # input-writing DMA, in block/instruction order (= schedule order)
        cum = {}
        req = {}   # input memref name -> {sem_id: (wait_entry_template, val)}
        for blk in func.get("blocks", []):
            for ins in blk.get("instructions", []):
                si = ins.get("sync_info") or {}
                for u in si.get("on_update") or []:
                    if u.get("sync_type") != "semaphore":
                        continue
                    sid = u["id"]
                    cum[sid] = cum.get(sid, 0) + int(u.get("update_value", 0))
                    if ins.get("opcode") == "DMACopy":
                        outs = ins.get("outs") or []
                        if outs and isinstance(outs[0], dict):
                            mref = outs[0].get("memref", "")
                            if mref.startswith("cc_") and mref.endswith("_in"):
                                req.setdefault(mref, {})[sid] = (u, cum[sid])
        # pass 2: rewrite collective waits
        for blk in func.get("blocks", []):
            for ins in blk.get("instructions", []):
                if ins.get("opcode") != "CollectiveCompute":
                    continue
                ins_aps = ins.get("ins") or []
                mref = ""
                if ins_aps and isinstance(ins_aps[0], dict):
                    mref = ins_aps[0].get("memref", "")
                si = ins.setdefault("sync_info", {"on_wait": [], "on_update": []})
                waits = []
                for sid, (u, val) in (req.get(mref) or {}).items():
                    waits.append({
                        "ant_name": u.get("ant_name", f"sem{sid}"),
                        "id": sid,
                        "sync_type": "semaphore",
                        "wait_mode": "sem-ge-imm",
                        "wait_value": val,
                    })
                si["on_wait"] = waits
                n += 1
    return n


def _patched_compile_bir_kernel(bir_json, tmpdir, neff_name="file.neff"):
    mod = json.loads(bir_json)
    changed = _fix_collective_waits(mod)
    changed += _split_waits(mod)
    if changed:
        bir_json = json.dumps(mod).encode()
    return _orig_compile_bir_kernel(bir_json, tmpdir, neff_name)


bass_utils.compile_bir_kernel = _patched_compile_bir_kernel
bass2jax.compile_bir_kernel = _patched_compile_bir_kernel


# ---------------------------------------------------------------------------
# kernel build
# ---------------------------------------------------------------------------
def _transpose_cast(nc, ps_pool, dst, src_ap, copy_engine, ident):
    """PE-transpose one [128,128] fp32 tile and cast-copy it into dst (bf16)."""
    pt = ps_pool.tile([P, P], F32, tag="tp")
    nc.tensor.transpose(pt[:], src_ap, ident[:])
    if copy_engine == "v":
        nc.vector.tensor_copy(out=dst, in_=pt[:])
    else:
        nc.scalar.copy(out=dst, in_=pt[:])


def build_nc():
    nc = bass.Bass(num_devices=N_CORES)

    Qp = nc.declare_dram_parameter("q_in", [NS, D], F32, isOutput=False)
    Kp = nc.declare_dram_parameter("k_in", [NS, D], F32, isOutput=False)
    Vp = nc.declare_dram_parameter("v_in", [NS, D], F32, isOutput=False)
    Wqp = nc.declare_dram_parameter("wq", [D, D], F32, isOutput=False)
    Wkp = nc.declare_dram_parameter("wk", [D, D], F32, isOutput=False)
    Wvp = nc.declare_dram_parameter("wv", [D, D], F32, isOutput=False)
    out_p = nc.declare_dram_parameter("out", [NS, D], F32, isOutput=True)

    # internal DRAM for collectives
    cc_k_in = nc.dram_tensor("cc_k_in", [DT, P, NS], BF16)
    cc_k_out = nc.dram_tensor("cc_k_out", [N_CORES, DT, P, NS], BF16,
                              addr_space="Shared")
    cc_v_in = nc.dram_tensor("cc_v_in", [NT, P, D], BF16)
    cc_v_out = nc.dram_tensor("cc_v_out", [N_CORES, NT, P, D], BF16,
                              addr_space="Shared")
    cc_warm_in = nc.dram_tensor("cc_warm_in", [1, 64], BF16)
    cc_warm_out = nc.dram_tensor("cc_warm_out", [N_CORES, 64], BF16,
                                 addr_space="Shared")

    with tile.TileContext(nc) as tc:
        with tc.tile_pool(name="persist", bufs=1) as pp:
            # tiny warm-up collective issued immediately: the first collective
            # pays a multi-10us comm-init barrier on the CC stream; absorb it
            # while the PE is busy with transposes so the real k-gather
            # starts without delay.
            nc.gpsimd.collective_compute(
                "AllGather", mybir.AluOpType.bypass,
                replica_groups=[list(range(N_CORES))],
                ins=[cc_warm_in[:]], outs=[cc_warm_out[:]],
            )
            ident = pp.tile([P, P], F32)
            make_identity(nc, ident[:])
            ones = pp.tile([P, 1], BF16)
            nc.vector.memset(ones[:], 1.0)
            # persistent bf16 arrays
            qT = pp.tile([P, DT, NS], BF16)      # q.T  [j, i]
            pT = pp.tile([P, LT, NS], BF16)      # softmax numerators [l, i]

            with tc.tile_pool(name="ps_stage", bufs=4, space="PSUM") as psst, \
                 tc.tile_pool(name="stage", bufs=3) as stg:

                def load_transpose(param, n_row_tiles, dst, eng):
                    # param: DRAM [n_row_tiles*P, D] fp32; dst bf16 tile
                    # [P, DT, n_row_tiles*P] holding param.T ([d, row]).
                    for rb in range(n_row_tiles):
                        s = stg.tile([P, D], F32, tag="ldw")
                        nc.sync.dma_start(
                            out=s[:], in_=param[rb * P:(rb + 1) * P, :]
                        )
                        for dt in range(DT):
                            _transpose_cast(
                                nc, psst,
                                dst[:, dt, rb * P:(rb + 1) * P],
                                s[:, dt * P:(dt + 1) * P],
                                eng, ident,
                            )

                # --- K branch first so its collective launches earliest ---
                wkT = stg.tile([P, DT, D], BF16, tag="wkT")
                load_transpose(Wkp, DT, wkT, "v")
                kTl = stg.tile([P, DT, NS], BF16, tag="kTl")
                load_transpose(Kp, NT, kTl, "s")
                # kT_loc[j, l_loc] = sum_d WkT[d, j-slice].T @ KT[d, l_loc]
                kTs = stg.tile([P, DT, NS], BF16, tag="kTs")
                for jt in range(DT):
                    pk = psst.tile([P, NS], F32, tag="mm")
                    for dt in range(DT):
                        nc.tensor.matmul(
                            pk[:],
                            wkT[:, dt, jt * P:(jt + 1) * P],
                            kTl[:, dt, :],
                            start=(dt == 0), stop=(dt == DT - 1),
                        )
                    nc.vector.tensor_copy(out=kTs[:, jt, :], in_=pk[:])
                    nc.gpsimd.dma_start(out=cc_k_in[jt], in_=kTs[:, jt, :])
                nc.gpsimd.collective_compute(
                    "AllGather", mybir.AluOpType.bypass,
                    replica_groups=[list(range(N_CORES))],
                    ins=[cc_k_in[:]], outs=[cc_k_out[:]],
                )

                # --- Q branch (local only; overlaps the k collective) ---
                wqT = stg.tile([P, DT, D], BF16, tag="wkT")
                load_transpose(Wqp, DT, wqT, "v")
                qTl = stg.tile([P, DT, NS], BF16, tag="kTl")
                load_transpose(Qp, NT, qTl, "s")
                for jt in range(DT):
                    pq = psst.tile([P, NS], F32, tag="mm")
                    for dt in range(DT):
                        nc.tensor.matmul(
                            pq[:],
                            wqT[:, dt, jt * P:(jt + 1) * P],
                            qTl[:, dt, :],
                            start=(dt == 0), stop=(dt == DT - 1),
                        )
                    nc.vector.tensor_copy(out=qT[:, jt, :], in_=pq[:])

                # --- V branch ---
                wvT = stg.tile([P, DT, D], BF16, tag="wkT")
                load_transpose(Wvp, DT, wvT, "v")
                vTl = stg.tile([P, DT, NS], BF16, tag="kTl")
                load_transpose(Vp, NT, vTl, "s")
                # v_loc[l_loc, m] = sum_d VT[d, l-slice].T @ WvT[d, m]
                vls = stg.tile([P, NT, D], BF16, tag="vls")
                for lt in range(NT):
                    for mh in range(2):
                        pv = psst.tile([P, NS], F32, tag="mm")
                        for dt in range(DT):
                            nc.tensor.matmul(
                                pv[:],
                                vTl[:, dt, lt * P:(lt + 1) * P],
                                wvT[:, dt, mh * NS:(mh + 1) * NS],
                                start=(dt == 0), stop=(dt == DT - 1),
                            )
                        nc.vector.tensor_copy(
                            out=vls[:, lt, mh * NS:(mh + 1) * NS], in_=pv[:]
                        )
                    nc.gpsimd.dma_start(out=cc_v_in[lt], in_=vls[:, lt, :])
                nc.gpsimd.collective_compute(
                    "AllGather", mybir.AluOpType.bypass,
                    replica_groups=[list(range(N_CORES))],
                    ins=[cc_v_in[:]], outs=[cc_v_out[:]],
                )

            # stage pool freed; bring in gathered k.T / v
            with tc.tile_pool(name="gathered", bufs=1) as gp, \
                 tc.tile_pool(name="ps_po", bufs=1, space="PSUM") as psm_po:
                kT = gp.tile([P, DT, N], BF16)    # k.T [j, l] full
                vF = gp.tile([P, LT, D], BF16)    # v  [l, m] full
                # rank-major so the scores for rank r's key block can start
                # as soon as that rank's slice has landed
                for r in range(N_CORES):
                    for jt in range(DT):
                        nc.sync.dma_start(
                            out=kT[:, jt, r * NS:(r + 1) * NS],
                            in_=cc_k_out[r, jt],
                        )
                for r in range(N_CORES):
                    for lt in range(NT):
                        nc.sync.dma_start(
                            out=vF[:, r * NT + lt, :],
                            in_=cc_v_out[r, lt],
                        )

                scale = float(1.0 / np.sqrt(D))
                po = [psm_po.tile([P, NS], F32, tag=f"po{it}", name=f"po{it}")
                      for it in range(NT)]
                unorm0 = gp.tile([P, NT, NS], F32)   # pass-A output (SBUF)

                # pass A: scores + exp for all l, out-matmuls for m-half 0.
                # NOTE: a matmul's start=True clears has_written for its whole
                # PSUM bank, so each concurrent accumulation chain must own
                # its own tile (tiles are padded to a bank).
                with tc.tile_pool(name="ps_sc", bufs=1, space="PSUM") as psm_sc:
                    for lt in range(LT):
                        ps = psm_sc.tile([P, NS], F32, tag="scores", bufs=2)
                        for jt in range(DT):
                            nc.tensor.matmul(
                                ps[:],
                                kT[:, jt, lt * P:(lt + 1) * P],
                                qT[:, jt, :],
                                start=(jt == 0), stop=(jt == DT - 1),
                            )
                        nc.scalar.activation(
                            out=pT[:, lt, :], in_=ps[:],
                            func=mybir.ActivationFunctionType.Exp, scale=scale,
                        )
                        for it in range(NT):
                            nc.tensor.matmul(
                                po[it][:],
                                pT[:, lt, it * P:(it + 1) * P],
                                vF[:, lt, 0:NS],
                                start=(lt == 0), stop=(lt == LT - 1),
                                skip_group_check=True,
                            )
                    # free po banks for pass B: park pass-A sums in SBUF
                    for it in range(NT):
                        nc.vector.tensor_copy(
                            out=unorm0[:, it, :], in_=po[it][:]
                        )

                # pass B: out-matmuls for m-half 1, denominator chains
                # interleaved (each chain owns a PSUM tile = its own bank,
                # and shares its stationary pT tile with the po2 matmul)
                with tc.tile_pool(name="ps_dn", bufs=1, space="PSUM") as psm_dn:
                    po2 = [psm_po.tile([P, NS], F32, tag=f"po{it}",
                                       name=f"po2_{it}")
                           for it in range(NT)]
                    dn = [psm_dn.tile([P, 1], F32, tag=f"dn{it}",
                                      name=f"dn{it}")
                          for it in range(NT)]
                    for lt in range(LT):
                        for it in range(NT):
                            nc.tensor.matmul(
                                po2[it][:],
                                pT[:, lt, it * P:(it + 1) * P],
                                vF[:, lt, NS:D],
                                start=(lt == 0), stop=(lt == LT - 1),
                                skip_group_check=True,
                            )
                            nc.tensor.matmul(
                                dn[it][:],
                                pT[:, lt, it * P:(it + 1) * P],
                                ones[:],
                                start=(lt == 0), stop=(lt == LT - 1),
                                skip_group_check=True,
                            )
                    dnr = gp.tile([P, NT], F32)
                    for it in range(NT):
                        nc.vector.tensor_copy(
                            out=dnr[:, it:it + 1], in_=dn[it][:]
                        )
                    rec = gp.tile([P, NT], F32)
                    nc.vector.reciprocal(out=rec[:], in_=dnr[:])

                    # normalize + store
                    for it in range(NT):
                        ob = gp.tile([P, NS], F32, tag="ob", bufs=2,
                                     name=f"ob0_{it}")
                        nc.vector.tensor_scalar_mul(
                            out=ob[:], in0=unorm0[:, it, :],
                            scalar1=rec[:, it:it + 1]
                        )
                        nc.sync.dma_start(
                            out=out_p[it * P:(it + 1) * P, 0:NS], in_=ob[:]
                        )
                        ob1 = gp.tile([P, NS], F32, tag="ob", bufs=2,
                                      name=f"ob1_{it}")
                        nc.vector.tensor_scalar_mul(
                            out=ob1[:], in0=po2[it][:],
                            scalar1=rec[:, it:it + 1]
                        )
                        nc.sync.dma_start(
                            out=out_p[it * P:(it + 1) * P, NS:D], in_=ob1[:]
                        )

    return nc


_nc_cache = None


def _get_nc():
    global _nc_cache
    if _nc_cache is None:
        _nc_cache = build_nc()
    return _nc_cache


def kernel(Q, K, V, Wq, Wk, Wv, _trace=False):
    from concourse.bass_utils import run_bass_kernel_spmd

    Q = np.ascontiguousarray(np.asarray(Q, dtype=np.float32))
    K = np.ascontiguousarray(np.asarray(K, dtype=np.float32))
    V = np.ascontiguousarray(np.asarray(V, dtype=np.float32))
    Wq = np.ascontiguousarray(np.asarray(Wq, dtype=np.float32))
    Wk = np.ascontiguousarray(np.asarray(Wk, dtype=np.float32))
    Wv = np.ascontiguousarray(np.asarray(Wv, dtype=np.float32))

    nc = _get_nc()
    in_maps = []
    for c in range(N_CORES):
        sl = slice(c * NS, (c + 1) * NS)
        in_maps.append({
            "q_in": Q[sl], "k_in": K[sl], "v_in": V[sl],
            "wq": Wq, "wk": Wk, "wv": Wv,
        })
    res = run_bass_kernel_spmd(
        nc, in_maps, list(range(N_CORES)), trace=_trace
    )
    out = np.concatenate([res.results[c]["out"] for c in range(N_CORES)], axis=0)
    if _trace:
        kernel.last_exec_time_ns = res.exec_time_ns
        kernel.last_results = res
    return out
